# revision 57
# baseline (speedup 1.0000x reference)
"""DisMax loss first part: logits = -(|s|*d + mean_c(|s|*d)) / temp, where
d[b,c] = ||fn_b - pn_c|| / sqrt(2) = sqrt(1 - cos(f_b, p_c)) for l2-normalized rows.

Single-pipeline design, data-parallel over batch on 8 cores; per core:
[1024, 512] x [512, 10000] -> [1024, 10000].

Key structure (vs the two-phase v0):
  - ONE activation-table set for the whole kernel (sqrt_and_others: Sqrt,
    Square, Abs, Copy).  Inverse prototype norms come from DVE
    reciprocal_approx_fast(norm^2) followed by ACT Sqrt -- NOT
    Abs_reciprocal_sqrt, whose table set would force a phase split or
    ~2.7us table swaps.
  - Prototype chunks (1000 classes) stream through a per-chunk pipeline:
    SWDGE cast-DMA load (f32->bf16), squares (ACT/DVE), ones-matmul class
    norm^2 (PE), recip approx (DVE), Sqrt (ACT), ones-broadcast (PE),
    normalize (DVE), cast to fp8 (GPSIMD/DVE).  Two batch tiles' GEMMs are
    emitted inside the chunk loop; the remaining six follow and the PE
    naturally runs ahead as chunks become ready, so no engine idles during
    the ~65us input stream.
  - Main GEMM in fp8e4 with DoubleRow (256-deep contraction per MM):
    2 MMs per [128 x 500] psum instead of 4 bf16 MMs.
  - ACT epilogue fuses scale into the sqrt: s = sqrt(c0^2 - c0^2*cos)
    = |c0|*d with c0 = -|scale|/temp (scale/bias are [128,1] APs computed
    on device), with fused row-sum accumulation for the mean.
  - dist is kept in fp16 (halves SBUF), pass-2 out = -(s + mean) is a DVE
    tensor_scalar at 4x rate, and the OUTPUT IS STORED AS FP16 (half the
    HBM write traffic; host upcasts to f32; adds ~2e-4 rel err).
"""

import sys
import types

for _p in ("/opt/trn_rl_repo", "/root/.axon_site"):
    if _p not in sys.path:
        sys.path.insert(0, _p)

# The NTFF profiling hook module is absent from this image's antenv package;
# inject the ctypes-based equivalent so trace=True works when requested.
if "antenv.axon_hooks" not in sys.modules:
    try:
        import trn_agent_boot.trn_boot as _tb

        _hook = _tb._ntff_profile_via_ctypes("/opt/axon/libaxon_pjrt.so")
        _m = types.ModuleType("antenv.axon_hooks")
        _m.get_axon_ntff_profile_hook = lambda: _hook
        sys.modules["antenv.axon_hooks"] = _m
    except Exception:
        pass

import numpy as np

import concourse.bacc as bacc
import concourse.tile as tile
import concourse.mybir as mybir
from concourse.bass_utils import run_bass_kernel_spmd

F32 = mybir.dt.float32
F32R = mybir.dt.float32r
BF16 = mybir.dt.bfloat16
FP16 = mybir.dt.float16
FP8 = mybir.dt.float8e4
ALU = mybir.AluOpType
ACTF = mybir.ActivationFunctionType

N_CORES = 8
B, C, D = 8192, 10000, 512
BPC = B // N_CORES          # 1024 batch rows per core
NB = BPC // 128             # 8 batch tiles
ND = D // 128               # 4 contraction subtiles of 128
CCH = 500                   # psum half-chunk (1 f32 PSUM bank)
BCH = 1000                  # class chunk
NCH = C // BCH              # 10 chunks
INLOOP = 1                  # batch tiles emitted inside the chunk loop
OQ = 2500                   # output store width (0.64 MB fp16 DMA)
NOQ = C // OQ

GEMM_FP8 = True             # fp8 DoubleRow halves PE time; the bf16->fp8
                            # conversion runs on ACT (1 elem/cycle measured;
                            # DVE/GPSIMD fp8 writes are 4-6x slower).

DR = mybir.MatmulPerfMode.DoubleRow


def build_nc():
    nc = bacc.Bacc("TRN2", target_bir_lowering=False, debug=False,
                   num_devices=N_CORES)
    f_h = nc.dram_tensor("f", [BPC, D], F32, kind="ExternalInput")
    pt_h = nc.dram_tensor("pt", [D, C], F32, kind="ExternalInput")
    s_h = nc.dram_tensor("s", [1, 2], F32, kind="ExternalInput")
    o_h = nc.dram_tensor("o", [BPC, C], FP16, kind="ExternalOutput")

    gdt = FP8 if GEMM_FP8 else BF16

    from contextlib import ExitStack

    with tile.TileContext(nc) as tc:
        with ExitStack() as stack:
            ep = stack.enter_context
            const_pool = ep(tc.tile_pool(name="const", bufs=1))
            persist_pool = ep(tc.tile_pool(name="persist", bufs=1))
            junk_pool = ep(tc.tile_pool(name="junk", bufs=1))
            pstage_pool = ep(tc.tile_pool(name="pstg", bufs=4))
            fstage_pool = ep(tc.tile_pool(name="fst", bufs=1))
            sq_pool = ep(tc.tile_pool(name="sq", bufs=2))
            ib_pool = ep(tc.tile_pool(name="ib", bufs=2))
            inv2_pool = ep(tc.tile_pool(name="inv2", bufs=1))
            norm_pool = ep(tc.tile_pool(name="norms", bufs=2))
            dist_pool = ep(tc.tile_pool(name="dist", bufs=3))
            # phase-A PSUM pools -- released before phase B so the 4-bank
            # FD-2000 epilogue psum tiles can double-buffer
            ps_c_pool = tc.alloc_tile_pool(name="ps_c", bufs=3, space="PSUM")
            ps_m_pool = tc.alloc_tile_pool(name="ps_m", bufs=1, space="PSUM")

            # persistent operands
            pnT = persist_pool.tile([128, ND, C], gdt, tag="pnT")
            fnT = persist_pool.tile([128, ND, BPC], gdt, tag="fnT")
            cb = persist_pool.tile([128, 4], F32, tag="cb")  # c0, c0^2, -c0^2
            rs = persist_pool.tile([128, NB, NCH], F32, tag="rs")

            ones_b = const_pool.tile([128, 1], BF16, tag="ones_b")
            nc.vector.memset(ones_b[:, :], 1.0)
            ones_f = const_pool.tile([1, 128], F32, tag="ones_f")
            nc.vector.memset(ones_f[:, :], 1.0)
            ones_bb = const_pool.tile([128, 128], BF16, tag="ones_bb")
            nc.vector.memset(ones_bb[:, :], 1.0)
            from concourse import masks

            ident = const_pool.tile([128, 128], BF16, tag="ident")
            masks.make_identity(nc, ident[:, :])
            dums = const_pool.tile([128, 512], BF16, tag="dums")
            nc.vector.memset(dums[:, :], 0.125)

            def pe_treadmill(n):
                """Dummy ones-matmuls emitted where the PE queue would
                otherwise idle: keeps the HAM activity window busy so the
                PE clock stays at 2.4 GHz through phase A (idle >3.4us
                re-throttles to 1.2 GHz and doubles every real matmul)."""
                pcd = ps_c_pool.tile([128, 2, 512], F32, tag="pc",
                                     name="pc_warm")
                for r in range(n):
                    nc.tensor.matmul(pcd[:1, r % 2, :], ones_b[:, :],
                                     dums[:, :], start=True, stop=True,
                                     skip_group_check=True)

            # chunk-0 prototype load + feature load go out first on the
            # SWDGE queue; everything else overlaps the HBM stream
            pt_r = pt_h[:, :].rearrange("(t p) c -> p t c", p=128)
            pstgs, ibs, sqbs = {}, {}, {}

            def load_chunk(c):
                pstg0 = pstage_pool.tile([128, ND, 1024], BF16, tag="pstg",
                                         name=f"pstg_{c}")
                pstgs[c] = pstg0[:, :, :BCH]
                nc.gpsimd.dma_start(pstgs[c],
                                    pt_r[:, :, c * BCH:(c + 1) * BCH])

            load_chunk(0)
            fst = fstage_pool.tile([128, NB, D], BF16, tag="fst")
            f_r = f_h[:, :].rearrange("(t p) d -> p t d", p=128)
            nc.gpsimd.dma_start(fst[:, :, :], f_r)  # casts f32 -> bf16

            # ---- scalar params: c0 = -|ds|/temp; need c0^2 and -c0^2 ------
            stile = const_pool.tile([1, 2], F32, tag="stile")
            nc.sync.dma_start(stile[:, :], s_h[:, :])
            srow4 = const_pool.tile([1, 4], F32, tag="srow4")
            nc.scalar.activation(srow4[:, 3:4], stile[:, 0:1], ACTF.Abs)
            nc.vector.reciprocal(srow4[:, 2:3], stile[:, 1:2])
            # c0 = -|ds| * (1/temp)
            nc.vector.scalar_tensor_tensor(srow4[:, 0:1], srow4[:, 3:4], -1.0,
                                           srow4[:, 2:3], op0=ALU.mult,
                                           op1=ALU.mult)
            # c0^2
            nc.vector.tensor_tensor(srow4[:, 1:2], srow4[:, 0:1],
                                    srow4[:, 0:1], op=ALU.mult)
            # -c0^2
            nc.vector.tensor_scalar(srow4[:, 2:3], srow4[:, 1:2], -1.0, None,
                                    op0=ALU.mult)
            ps_b = ps_m_pool.tile([128, 2, 512], F32, tag="m", name="ps_scal")
            nc.tensor.matmul(ps_b[:, 0, :4], ones_f[:, :], srow4[:, :],
                             start=True, stop=True)
            nc.vector.tensor_copy(cb[:, :], ps_b[:, 0, :4])

            # ---- feature prep: fnT[p, d, b] = (f/||f||)[b, 128*d+p] -------
            # Only the ACT squares are emitted up front; the finv chain and
            # per-tile transposes are spread through the first chunk
            # iterations so they never block the chunk pipeline's queues.
            fss = norm_pool.tile([128, NB], F32, tag="fss")
            finv = norm_pool.tile([128, NB], F32, tag="finv")
            for t in range(NB):
                junk = junk_pool.tile([128, D], BF16, tag="junk")
                nc.vector.scalar_tensor_tensor(junk[:, :], fst[:, t, :], 1.0,
                                               fst[:, t, :], op0=ALU.mult,
                                               op1=ALU.mult,
                                               accum_out=fss[:, t:t + 1])

            def feature_finv():
                nc.vector.reciprocal_approx_fast(out=finv[:, :],
                                                 in_=fss[:, :])
                nc.scalar.activation(finv[:, :], finv[:, :], ACTF.Sqrt)

            def feature_tile(t):
                fbf = junk_pool.tile([128, D], BF16, tag="junk", name="fbf")
                nc.vector.tensor_scalar(fbf[:, :], fst[:, t, :],
                                        finv[:, t:t + 1], None, op0=ALU.mult)
                ps_t0 = ps_m_pool.tile([128, 2, 512], F32, tag="m",
                                       name="ps_tr")
                ps_t = (ps_t0[:, 0, :].bitcast(BF16)[:, :ND * 128]
                        .rearrange("p (d c) -> p d c", d=ND))
                for d in range(ND):
                    nc.tensor.transpose(ps_t[:, d, :],
                                        fbf[:, d * 128:(d + 1) * 128],
                                        ident[:, :])
                if GEMM_FP8:
                    # ACT converts to fp8 at line rate (DVE would be ~6x)
                    nc.scalar.activation(fnT[:, :, t * 128:(t + 1) * 128],
                                         ps_t[:, :, :], ACTF.Copy)
                else:
                    nc.vector.tensor_copy(fnT[:, :, t * 128:(t + 1) * 128],
                                          ps_t[:, :, :])

            FEAT_SCHED = {2: (feature_finv, [0, 1, 2]), 3: (None, [3, 4]),
                          4: (None, [5, 6]), 5: (None, [7])}

            # ---- helpers ---------------------------------------------------
            def gemm_chunk(i, c):
                """Emit the GEMM + sqrt-epilogue for batch tile i, chunk c."""
                cc = c * BCH
                pc = ps_c_pool.tile([128, 2, 512], F32, tag="pc")
                # d outer / h inner: each weight load serves both psum halves
                if GEMM_FP8:
                    for k in range(2):
                        for h in range(2):
                            nc.tensor.matmul(
                                pc[:, h, :CCH],
                                fnT[:, 2 * k:2 * k + 2,
                                    i * 128:(i + 1) * 128],
                                pnT[:, 2 * k:2 * k + 2,
                                    cc + h * CCH:cc + h * CCH + CCH],
                                start=(k == 0), stop=(k == 1), perf_mode=DR,
                                skip_group_check=True)
                else:
                    for d in range(ND):
                        for h in range(2):
                            nc.tensor.matmul(
                                pc[:, h, :CCH],
                                fnT[:, d, i * 128:(i + 1) * 128],
                                pnT[:, d, cc + h * CCH:cc + h * CCH + CCH],
                                start=(d == 0), stop=(d == ND - 1),
                                skip_group_check=True)
                dist = dists[i]
                dv = dist[:, cc:cc + BCH].rearrange("p (h c) -> p h c", h=2)
                # s = sqrt(c0^2 - c0^2 * cos) = |c0| * d;  rowsum for mean
                nc.scalar.activation(dv, pc[:, :, :CCH], ACTF.Sqrt,
                                     bias=cb[:, 1:2], scale=cb[:, 2:3],
                                     accum_out=rs[:, i, c:c + 1])

            def finish_tile(i, nslots=NCH):
                """Row mean + pass-2 + store for batch tile i."""
                rsum = norm_pool.tile([128, 1], F32, tag="rsum")
                bvec = norm_pool.tile([128, 1], F32, tag="bvec")
                nc.vector.reduce_sum(rsum[:, :], rs[:, i, :nslots],
                                     axis=mybir.AxisListType.X)
                nc.vector.tensor_scalar(bvec[:, :], rsum[:, :], 1.0 / C, None,
                                        op0=ALU.mult)
                dist = dists[i]
                for q in range(NOQ):
                    # output staged through the (idle-by-now) pstg pool
                    ob0 = pstage_pool.tile([128, ND, 1024], BF16,
                                           tag="pstg", name=f"ob_{i}_{q}")
                    ob = (ob0[:, :, :].rearrange("p a b -> p (a b)")
                          .bitcast(FP16)[:, :OQ])
                    # out = -(s + mean) = c0*d + c0*mean(d)
                    nc.vector.tensor_scalar(ob,
                                            dist[:, q * OQ:(q + 1) * OQ],
                                            bvec[:, 0:1], -1.0,
                                            op0=ALU.add, op1=ALU.mult)
                    nc.sync.dma_start(
                        o_h[i * 128:(i + 1) * 128, q * OQ:(q + 1) * OQ],
                        ob)

            dists = {}
            for i in range(NB):
                dists[i] = dist_pool.tile([128, C], FP16, tag="dist",
                                          name=f"dist_{i}")

            # ---- prototype chunk pipeline + in-loop GEMM tiles -------------
            # Deep modulo schedule; per iteration k:
            #   load(k) | squares(k-1) | gram(k-1) | GEMMs(k-3) |
            #   normalize(k-2) | recip(k-1) | ib=sqrt(k-1)
            # gram = all-ones [128,128] matmul: computes the class norm^2
            # partition-reduction ALREADY BROADCAST across partitions, so
            # recip/sqrt run 128-lane-wide and feed the normalize directly
            # (no row ops, no separate broadcast matmul).  Each engine's
            # in-order queue only sees ops whose deps resolved in earlier
            # iterations, so nothing stalls on the in-flight norm chain.
            for k in range(1, NCH + 3):
                if k < NCH:
                    load_chunk(k)
                # in-loop GEMMs on chunk k-3 FIRST in the PE queue: they
                # have no same-iteration deps, so the PE never stalls
                if k - 3 >= 0:
                    for i in range(INLOOP):
                        gemm_chunk(i, k - 3)
                if 0 <= k - 1 < NCH:
                    # squares, h-major strided (2 DVE TT at 2x rate)
                    pstg = pstgs[k - 1]
                    sqb = sqbs[k - 1] = sq_pool.tile([128, ND, BCH], BF16,
                                                     tag="sq",
                                                     name=f"sq_{k - 1}")
                    for h in range(2):
                        nc.vector.tensor_tensor(
                            sqb[:, :, h * CCH:(h + 1) * CCH],
                            pstg[:, :, h * CCH:(h + 1) * CCH],
                            pstg[:, :, h * CCH:(h + 1) * CCH], op=ALU.mult)
                    # broadcast norm^2: ones[128,128] x squares
                    pb = ps_m_pool.tile([128, 2, 512], F32, tag="m",
                                        name="pb")
                    for h in range(2):
                        for d in range(ND):
                            nc.tensor.matmul(pb[:, h, :CCH], ones_bb[:, :],
                                             sqb[:, d, h * CCH:(h + 1) * CCH],
                                             start=(d == 0),
                                             stop=(d == ND - 1))
                # normalize chunk k-2 (DVE TT 2x); fp8 mode overwrites the
                # squares tile and lets ACT cast it into pnT at line rate
                if 0 <= k - 2 < NCH:
                    cp = (k - 2) * BCH
                    if GEMM_FP8:
                        for d in range(ND):
                            nc.vector.tensor_tensor(sqbs[k - 2][:, d, :],
                                                    pstgs[k - 2][:, d, :],
                                                    ibs[k - 2][:, :],
                                                    op=ALU.mult)
                        nc.scalar.activation(pnT[:, :, cp:cp + BCH],
                                             sqbs[k - 2][:, :, :],
                                             ACTF.Copy)
                    else:
                        for d in range(ND):
                            nc.vector.tensor_tensor(pnT[:, d, cp:cp + BCH],
                                                    pstgs[k - 2][:, d, :],
                                                    ibs[k - 2][:, :],
                                                    op=ALU.mult)
                if 0 <= k - 1 < NCH:
                    # 1/||p|| broadcast = Sqrt(recip_approx(norm^2)) -- the
                    # sqrt stays on the main loop's ACT table set
                    inv2 = inv2_pool.tile([128, 2, CCH], F32, tag="inv2")
                    nc.vector.reciprocal_approx_fast(out=inv2[:, :, :],
                                                     in_=pb[:, :, :CCH])
                    ib = ib_pool.tile([128, BCH], BF16, tag="ib",
                                      name=f"ib_{k - 1}")
                    ibs[k - 1] = ib
                    nc.scalar.activation(
                        ib[:, :].rearrange("p (h c) -> p h c", h=2),
                        inv2[:, :, :], ACTF.Sqrt)
                # keep the PE clock warm through the pipeline-fill iterations
                if k < 4:
                    pe_treadmill(10)
                # spread-out feature prep (fnT tiles 0-2 ready before the
                # first in-loop GEMMs at k=3)
                if k in FEAT_SCHED:
                    fn, tiles = FEAT_SCHED[k]
                    if fn is not None:
                        fn()
                    for t in tiles:
                        feature_tile(t)

            for i in range(INLOOP):
                finish_tile(i)

            # ---- remaining batch tiles (PE runs ahead as chunks land) ------
            if GEMM_FP8:
                # free the phase-A psum pools; phase B double-buffers 4-bank
                # psum tiles so the sqrt epilogue runs at FD=2000
                ps_m_pool.release()
                ps_c_pool.release()
                ps4_pool = tc.alloc_tile_pool(name="ps4", bufs=2,
                                              space="PSUM")
                for i in range(INLOOP, NB):
                    for j in range(NCH // 2):
                        pc4 = ps4_pool.tile([128, 4, 512], F32, tag="p4")
                        for q in range(4):
                            c5 = 2 * j * BCH + q * CCH
                            for g in range(2):
                                nc.tensor.matmul(
                                    pc4[:, q, :CCH],
                                    fnT[:, 2 * g:2 * g + 2,
                                        i * 128:(i + 1) * 128],
                                    pnT[:, 2 * g:2 * g + 2, c5:c5 + CCH],
                                    start=(g == 0), stop=(g == 1),
                                    perf_mode=DR, skip_group_check=True)
                        dist = dists[i]
                        dv = (dist[:, 2 * j * BCH:(2 * j + 2) * BCH]
                              .rearrange("p (q c) -> p q c", q=4))
                        nc.scalar.activation(dv, pc4[:, :, :CCH], ACTF.Sqrt,
                                             bias=cb[:, 1:2],
                                             scale=cb[:, 2:3],
                                             accum_out=rs[:, i, j:j + 1])
                    finish_tile(i, nslots=NCH // 2)
                ps4_pool.release()
            else:
                for i in range(INLOOP, NB):
                    for c in range(NCH):
                        gemm_chunk(i, c)
                    finish_tile(i)

    nc.compile()
    return nc


_CACHE = {}


def _get_nc():
    if "nc" not in _CACHE:
        _CACHE["nc"] = build_nc()
    return _CACHE["nc"]


def make_in_maps(features, prototypes, distance_scale, temperature):
    f = np.ascontiguousarray(np.asarray(features, dtype=np.float32))
    pt = np.ascontiguousarray(np.asarray(prototypes, dtype=np.float32).T)
    s = np.array([[np.float32(np.asarray(distance_scale).reshape(-1)[0]),
                   np.float32(np.asarray(temperature).reshape(-1)[0])]],
                 dtype=np.float32)
    return [
        {"f": f[i * BPC:(i + 1) * BPC], "pt": pt, "s": s}
        for i in range(N_CORES)
    ]


def run(features, prototypes, distance_scale, temperature, **kwargs):
    nc = _get_nc()
    in_maps = make_in_maps(features, prototypes, distance_scale, temperature)
    res = run_bass_kernel_spmd(nc, in_maps, core_ids=list(range(N_CORES)),
                               **kwargs)
    out = np.concatenate(
        [np.asarray(res.results[i]["o"]).astype(np.float32)
         for i in range(N_CORES)], axis=0)
    return out, res


def kernel(features, prototypes, distance_scale, temperature):
    out, _ = run(features, prototypes, distance_scale, temperature)
    return out


# revision 58
# speedup vs baseline: 1.0208x; 1.0208x over previous
"""DisMax loss first part: logits = -(|s|*d + mean_c(|s|*d)) / temp, where
d[b,c] = ||fn_b - pn_c|| / sqrt(2) = sqrt(1 - cos(f_b, p_c)) for l2-normalized rows.

Single-pipeline design, data-parallel over batch on 8 cores; per core:
[1024, 512] x [512, 10000] -> [1024, 10000].

Key structure (vs the two-phase v0):
  - ONE activation-table set for the whole kernel (sqrt_and_others: Sqrt,
    Square, Abs, Copy).  Inverse prototype norms come from DVE
    reciprocal_approx_fast(norm^2) followed by ACT Sqrt -- NOT
    Abs_reciprocal_sqrt, whose table set would force a phase split or
    ~2.7us table swaps.
  - Prototype chunks (1000 classes) stream through a per-chunk pipeline:
    SWDGE cast-DMA load (f32->bf16), squares (ACT/DVE), ones-matmul class
    norm^2 (PE), recip approx (DVE), Sqrt (ACT), ones-broadcast (PE),
    normalize (DVE), cast to fp8 (GPSIMD/DVE).  Two batch tiles' GEMMs are
    emitted inside the chunk loop; the remaining six follow and the PE
    naturally runs ahead as chunks become ready, so no engine idles during
    the ~65us input stream.
  - Main GEMM in fp8e4 with DoubleRow (256-deep contraction per MM):
    2 MMs per [128 x 500] psum instead of 4 bf16 MMs.
  - ACT epilogue fuses scale into the sqrt: s = sqrt(c0^2 - c0^2*cos)
    = |c0|*d with c0 = -|scale|/temp (scale/bias are [128,1] APs computed
    on device), with fused row-sum accumulation for the mean.
  - dist is kept in fp16 (halves SBUF), pass-2 out = -(s + mean) is a DVE
    tensor_scalar at 4x rate, and the OUTPUT IS STORED AS FP16 (half the
    HBM write traffic; host upcasts to f32; adds ~2e-4 rel err).
"""

import sys
import types

for _p in ("/opt/trn_rl_repo", "/root/.axon_site"):
    if _p not in sys.path:
        sys.path.insert(0, _p)

# The NTFF profiling hook module is absent from this image's antenv package;
# inject the ctypes-based equivalent so trace=True works when requested.
if "antenv.axon_hooks" not in sys.modules:
    try:
        import trn_agent_boot.trn_boot as _tb

        _hook = _tb._ntff_profile_via_ctypes("/opt/axon/libaxon_pjrt.so")
        _m = types.ModuleType("antenv.axon_hooks")
        _m.get_axon_ntff_profile_hook = lambda: _hook
        sys.modules["antenv.axon_hooks"] = _m
    except Exception:
        pass

import numpy as np

import concourse.bacc as bacc
import concourse.tile as tile
import concourse.mybir as mybir
from concourse.bass_utils import run_bass_kernel_spmd

F32 = mybir.dt.float32
F32R = mybir.dt.float32r
BF16 = mybir.dt.bfloat16
FP16 = mybir.dt.float16
FP8 = mybir.dt.float8e4
ALU = mybir.AluOpType
ACTF = mybir.ActivationFunctionType

N_CORES = 8
B, C, D = 8192, 10000, 512
BPC = B // N_CORES          # 1024 batch rows per core
NB = BPC // 128             # 8 batch tiles
ND = D // 128               # 4 contraction subtiles of 128
CCH = 500                   # psum half-chunk (1 f32 PSUM bank)
BCH = 1000                  # class chunk
NCH = C // BCH              # 10 chunks
INLOOP = 3                  # batch tiles emitted inside the chunk loop
OQ = 2500                   # output store width (0.64 MB fp16 DMA)
NOQ = C // OQ

GEMM_FP8 = False            # fp8 DoubleRow halves PE time but the bf16->fp8
                            # conversion costs ~6us/chunk on DVE (measured) --
                            # net loss; bf16 needs no conversion pass at all.

DR = mybir.MatmulPerfMode.DoubleRow


def build_nc():
    nc = bacc.Bacc("TRN2", target_bir_lowering=False, debug=False,
                   num_devices=N_CORES)
    f_h = nc.dram_tensor("f", [BPC, D], F32, kind="ExternalInput")
    pt_h = nc.dram_tensor("pt", [D, C], F32, kind="ExternalInput")
    s_h = nc.dram_tensor("s", [1, 2], F32, kind="ExternalInput")
    o_h = nc.dram_tensor("o", [BPC, C], FP16, kind="ExternalOutput")

    gdt = FP8 if GEMM_FP8 else BF16

    from contextlib import ExitStack

    with tile.TileContext(nc) as tc:
        with ExitStack() as stack:
            ep = stack.enter_context
            const_pool = ep(tc.tile_pool(name="const", bufs=1))
            persist_pool = ep(tc.tile_pool(name="persist", bufs=1))
            junk_pool = ep(tc.tile_pool(name="junk", bufs=1))
            pstage_pool = ep(tc.tile_pool(name="pstg", bufs=4))
            fstage_pool = ep(tc.tile_pool(name="fst", bufs=1))
            sq_pool = ep(tc.tile_pool(name="sq", bufs=1))
            ib_pool = ep(tc.tile_pool(name="ib", bufs=2))
            inv2_pool = ep(tc.tile_pool(name="inv2", bufs=1))
            norm_pool = ep(tc.tile_pool(name="norms", bufs=2))
            dist_pool = ep(tc.tile_pool(name="dist", bufs=3))
            ps_c_pool = ep(tc.tile_pool(name="ps_c", bufs=3, space="PSUM"))
            ps_m_pool = ep(tc.tile_pool(name="ps_m", bufs=1, space="PSUM"))

            # persistent operands
            pnT = persist_pool.tile([128, ND, C], gdt, tag="pnT")
            fnT = persist_pool.tile([128, ND, BPC], gdt, tag="fnT")
            cb = persist_pool.tile([128, 4], F32, tag="cb")  # c0, c0^2, -c0^2
            rs = persist_pool.tile([128, NB, NCH], F32, tag="rs")

            ones_b = const_pool.tile([128, 1], BF16, tag="ones_b")
            nc.vector.memset(ones_b[:, :], 1.0)
            ones_f = const_pool.tile([1, 128], F32, tag="ones_f")
            nc.vector.memset(ones_f[:, :], 1.0)
            ones_bb = const_pool.tile([128, 128], BF16, tag="ones_bb")
            nc.vector.memset(ones_bb[:, :], 1.0)
            from concourse import masks

            ident = const_pool.tile([128, 128], BF16, tag="ident")
            masks.make_identity(nc, ident[:, :])
            dums = const_pool.tile([128, 512], BF16, tag="dums")
            nc.vector.memset(dums[:, :], 0.125)

            def pe_treadmill(n):
                """Dummy ones-matmuls emitted where the PE queue would
                otherwise idle: keeps the HAM activity window busy so the
                PE clock stays at 2.4 GHz through phase A (idle >3.4us
                re-throttles to 1.2 GHz and doubles every real matmul)."""
                pcd = ps_c_pool.tile([128, 2, 512], F32, tag="pc",
                                     name="pc_warm")
                for r in range(n):
                    nc.tensor.matmul(pcd[:1, r % 2, :], ones_b[:, :],
                                     dums[:, :], start=True, stop=True,
                                     skip_group_check=True)

            # chunk-0 prototype load + feature load go out first on the
            # SWDGE queue; everything else overlaps the HBM stream
            pt_r = pt_h[:, :].rearrange("(t p) c -> p t c", p=128)
            pstgs, ibs = {}, {}

            def load_chunk(c):
                pstg0 = pstage_pool.tile([128, ND, 1024], BF16, tag="pstg",
                                         name=f"pstg_{c}")
                pstgs[c] = pstg0[:, :, :BCH]
                nc.gpsimd.dma_start(pstgs[c],
                                    pt_r[:, :, c * BCH:(c + 1) * BCH])

            load_chunk(0)
            fst = fstage_pool.tile([128, NB, D], BF16, tag="fst")
            f_r = f_h[:, :].rearrange("(t p) d -> p t d", p=128)
            nc.gpsimd.dma_start(fst[:, :, :], f_r)  # casts f32 -> bf16

            # ---- scalar params: c0 = -|ds|/temp; need c0^2 and -c0^2 ------
            stile = const_pool.tile([1, 2], F32, tag="stile")
            nc.sync.dma_start(stile[:, :], s_h[:, :])
            srow4 = const_pool.tile([1, 4], F32, tag="srow4")
            nc.scalar.activation(srow4[:, 3:4], stile[:, 0:1], ACTF.Abs)
            nc.vector.reciprocal(srow4[:, 2:3], stile[:, 1:2])
            # c0 = -|ds| * (1/temp)
            nc.vector.scalar_tensor_tensor(srow4[:, 0:1], srow4[:, 3:4], -1.0,
                                           srow4[:, 2:3], op0=ALU.mult,
                                           op1=ALU.mult)
            # c0^2
            nc.vector.tensor_tensor(srow4[:, 1:2], srow4[:, 0:1],
                                    srow4[:, 0:1], op=ALU.mult)
            # -c0^2
            nc.vector.tensor_scalar(srow4[:, 2:3], srow4[:, 1:2], -1.0, None,
                                    op0=ALU.mult)
            ps_b = ps_m_pool.tile([128, 2, 512], F32, tag="m", name="ps_scal")
            nc.tensor.matmul(ps_b[:, 0, :4], ones_f[:, :], srow4[:, :],
                             start=True, stop=True)
            nc.vector.tensor_copy(cb[:, :], ps_b[:, 0, :4])

            # ---- feature prep: fnT[p, d, b] = (f/||f||)[b, 128*d+p] -------
            # Only the ACT squares are emitted up front; the finv chain and
            # per-tile transposes are spread through the first chunk
            # iterations so they never block the chunk pipeline's queues.
            fss = norm_pool.tile([128, NB], F32, tag="fss")
            finv = norm_pool.tile([128, NB], F32, tag="finv")
            for t in range(NB):
                junk = junk_pool.tile([128, D], BF16, tag="junk")
                nc.scalar.activation(junk[:, :], fst[:, t, :], ACTF.Square,
                                     accum_out=fss[:, t:t + 1])

            def feature_finv():
                nc.vector.reciprocal_approx_fast(out=finv[:, :],
                                                 in_=fss[:, :])
                nc.scalar.activation(finv[:, :], finv[:, :], ACTF.Sqrt)

            def feature_tile(t):
                fbf = junk_pool.tile([128, D], BF16, tag="junk", name="fbf")
                nc.vector.tensor_scalar(fbf[:, :], fst[:, t, :],
                                        finv[:, t:t + 1], None, op0=ALU.mult)
                ps_t0 = ps_m_pool.tile([128, 2, 512], F32, tag="m",
                                       name="ps_tr")
                ps_t = (ps_t0[:, 0, :].bitcast(BF16)[:, :ND * 128]
                        .rearrange("p (d c) -> p d c", d=ND))
                for d in range(ND):
                    nc.tensor.transpose(ps_t[:, d, :],
                                        fbf[:, d * 128:(d + 1) * 128],
                                        ident[:, :])
                nc.vector.tensor_copy(fnT[:, :, t * 128:(t + 1) * 128],
                                      ps_t[:, :, :])

            FEAT_SCHED = {2: (feature_finv, [0, 1, 2]), 3: (None, [3, 4]),
                          4: (None, [5, 6]), 5: (None, [7])}

            # ---- helpers ---------------------------------------------------
            def gemm_chunk(i, c):
                """Emit the GEMM + sqrt-epilogue for batch tile i, chunk c."""
                cc = c * BCH
                pc = ps_c_pool.tile([128, 2, 512], F32, tag="pc")
                # d outer / h inner: each weight load serves both psum halves
                if GEMM_FP8:
                    for k in range(2):
                        for h in range(2):
                            nc.tensor.matmul(
                                pc[:, h, :CCH],
                                fnT[:, 2 * k:2 * k + 2,
                                    i * 128:(i + 1) * 128],
                                pnT[:, 2 * k:2 * k + 2,
                                    cc + h * CCH:cc + h * CCH + CCH],
                                start=(k == 0), stop=(k == 1), perf_mode=DR,
                                skip_group_check=True)
                else:
                    for d in range(ND):
                        for h in range(2):
                            nc.tensor.matmul(
                                pc[:, h, :CCH],
                                fnT[:, d, i * 128:(i + 1) * 128],
                                pnT[:, d, cc + h * CCH:cc + h * CCH + CCH],
                                start=(d == 0), stop=(d == ND - 1),
                                skip_group_check=True)
                dist = dists[i]
                dv = dist[:, cc:cc + BCH].rearrange("p (h c) -> p h c", h=2)
                # s = sqrt(c0^2 - c0^2 * cos) = |c0| * d;  rowsum for mean
                nc.scalar.activation(dv, pc[:, :, :CCH], ACTF.Sqrt,
                                     bias=cb[:, 1:2], scale=cb[:, 2:3],
                                     accum_out=rs[:, i, c:c + 1])

            def finish_tile(i):
                """Row mean + pass-2 + store for batch tile i."""
                rsum = norm_pool.tile([128, 1], F32, tag="rsum")
                bvec = norm_pool.tile([128, 1], F32, tag="bvec")
                nc.vector.reduce_sum(rsum[:, :], rs[:, i, :],
                                     axis=mybir.AxisListType.X)
                nc.vector.tensor_scalar(bvec[:, :], rsum[:, :], 1.0 / C, None,
                                        op0=ALU.mult)
                dist = dists[i]
                for q in range(NOQ):
                    # output staged through the (idle-by-now) pstg pool
                    ob0 = pstage_pool.tile([128, ND, 1024], BF16,
                                           tag="pstg", name=f"ob_{i}_{q}")
                    ob = (ob0[:, :, :].rearrange("p a b -> p (a b)")
                          .bitcast(FP16)[:, :OQ])
                    # out = -(s + mean) = c0*d + c0*mean(d)
                    nc.vector.tensor_scalar(ob,
                                            dist[:, q * OQ:(q + 1) * OQ],
                                            bvec[:, 0:1], -1.0,
                                            op0=ALU.add, op1=ALU.mult)
                    nc.sync.dma_start(
                        o_h[i * 128:(i + 1) * 128, q * OQ:(q + 1) * OQ],
                        ob)

            dists = {}
            for i in range(NB):
                dists[i] = dist_pool.tile([128, C], FP16, tag="dist",
                                          name=f"dist_{i}")

            # ---- prototype chunk pipeline + in-loop GEMM tiles -------------
            # Deep modulo schedule; per iteration k:
            #   load(k) | squares(k-1) | gram(k-1) | GEMMs(k-3) |
            #   normalize(k-2) | recip(k-1) | ib=sqrt(k-1)
            # gram = all-ones [128,128] matmul: computes the class norm^2
            # partition-reduction ALREADY BROADCAST across partitions, so
            # recip/sqrt run 128-lane-wide and feed the normalize directly
            # (no row ops, no separate broadcast matmul).  Each engine's
            # in-order queue only sees ops whose deps resolved in earlier
            # iterations, so nothing stalls on the in-flight norm chain.
            for k in range(1, NCH + 3):
                if k < NCH:
                    load_chunk(k)
                # in-loop GEMMs on chunk k-3 FIRST in the PE queue: they
                # have no same-iteration deps, so the PE never stalls
                if k - 3 >= 0:
                    for i in range(INLOOP):
                        gemm_chunk(i, k - 3)
                if 0 <= k - 1 < NCH:
                    # squares, h-major strided (2 DVE TT at 2x rate)
                    pstg = pstgs[k - 1]
                    sqb = sq_pool.tile([128, ND, BCH], BF16, tag="sq")
                    for h in range(2):
                        nc.vector.tensor_tensor(
                            sqb[:, :, h * CCH:(h + 1) * CCH],
                            pstg[:, :, h * CCH:(h + 1) * CCH],
                            pstg[:, :, h * CCH:(h + 1) * CCH], op=ALU.mult)
                    # broadcast norm^2: ones[128,128] x squares
                    pb = ps_m_pool.tile([128, 2, 512], F32, tag="m",
                                        name="pb")
                    for h in range(2):
                        for d in range(ND):
                            nc.tensor.matmul(pb[:, h, :CCH], ones_bb[:, :],
                                             sqb[:, d, h * CCH:(h + 1) * CCH],
                                             start=(d == 0),
                                             stop=(d == ND - 1))
                # normalize chunk k-2 straight into pnT (bf16, DVE TT 2x)
                if 0 <= k - 2 < NCH:
                    cp = (k - 2) * BCH
                    for d in range(ND):
                        nc.vector.tensor_tensor(pnT[:, d, cp:cp + BCH],
                                                pstgs[k - 2][:, d, :],
                                                ibs[k - 2][:, :],
                                                op=ALU.mult)
                if 0 <= k - 1 < NCH:
                    # 1/||p|| broadcast = Sqrt(recip_approx(norm^2)) -- the
                    # sqrt stays on the main loop's ACT table set
                    inv2 = inv2_pool.tile([128, 2, CCH], F32, tag="inv2")
                    nc.vector.reciprocal_approx_fast(out=inv2[:, :, :],
                                                     in_=pb[:, :, :CCH])
                    ib = ib_pool.tile([128, BCH], BF16, tag="ib",
                                      name=f"ib_{k - 1}")
                    ibs[k - 1] = ib
                    nc.scalar.activation(
                        ib[:, :].rearrange("p (h c) -> p h c", h=2),
                        inv2[:, :, :], ACTF.Sqrt)
                # keep the PE clock warm through the pipeline-fill iterations
                if k < 4:
                    pe_treadmill(10)
                # spread-out feature prep (fnT tiles 0-2 ready before the
                # first in-loop GEMMs at k=3)
                if k in FEAT_SCHED:
                    fn, tiles = FEAT_SCHED[k]
                    if fn is not None:
                        fn()
                    for t in tiles:
                        feature_tile(t)

            for i in range(INLOOP):
                finish_tile(i)

            # ---- remaining batch tiles (PE runs ahead as chunks land) ------
            for i in range(INLOOP, NB):
                for c in range(NCH):
                    gemm_chunk(i, c)
                finish_tile(i)

    nc.compile()
    return nc


_CACHE = {}


def _get_nc():
    if "nc" not in _CACHE:
        _CACHE["nc"] = build_nc()
    return _CACHE["nc"]


def make_in_maps(features, prototypes, distance_scale, temperature):
    f = np.ascontiguousarray(np.asarray(features, dtype=np.float32))
    pt = np.ascontiguousarray(np.asarray(prototypes, dtype=np.float32).T)
    s = np.array([[np.float32(np.asarray(distance_scale).reshape(-1)[0]),
                   np.float32(np.asarray(temperature).reshape(-1)[0])]],
                 dtype=np.float32)
    return [
        {"f": f[i * BPC:(i + 1) * BPC], "pt": pt, "s": s}
        for i in range(N_CORES)
    ]


def run(features, prototypes, distance_scale, temperature, **kwargs):
    nc = _get_nc()
    in_maps = make_in_maps(features, prototypes, distance_scale, temperature)
    res = run_bass_kernel_spmd(nc, in_maps, core_ids=list(range(N_CORES)),
                               **kwargs)
    out = np.concatenate(
        [np.asarray(res.results[i]["o"]).astype(np.float32)
         for i in range(N_CORES)], axis=0)
    return out, res


def kernel(features, prototypes, distance_scale, temperature):
    out, _ = run(features, prototypes, distance_scale, temperature)
    return out
